# revision 1
# baseline (speedup 1.0000x reference)
"""Trainium2 Bass kernel for the didgeridoo (conical bore) input-impedance model.

Math (matches the reference): for each integer frequency f in [fmin, fmax),
chain-multiply 128 per-slice lossy transmission-line 2x2 complex matrices
    T_n = [[ch_n, Z0_n*sh_n], [sh_n/Z0_n, ch_n]],   gamma_n = (alpha_n + i*k)*dL
then Ze = (A*ZL + B)/(C*ZL + D) against the unflanged-open-end radiation
impedance ZL, output |Ze|.

Kernel strategy (per the sharding hint): frequencies are sharded 8 ways
across cores (47 per core, padded), each core puts its frequencies on the
SBUF partition axis and the 128 bore slices on the free axis. The ordered
matrix product is a binary tree (7 levels) over planes packed re|im x
(A,B,C,D) in one tile: per level 8 strided tensor-multiplies (split
Vector/GPSIMD) write a term-interleaved tile and ONE tensor_reduce(add)
over the innermost 4 yields the next level; real-part sign flips come
from an ACT-negated imag half. The radiation load ZL is folded into the
last slice matrix so the tail is just |A|/|C|. cosh/sinh/cos/sin
arguments are all < 0.07 here, so fp32-exact Taylor polynomials replace
transcendentals.
"""
import math
from contextlib import ExitStack

import numpy as np

import concourse.bass as bass
import concourse.bacc as bacc
import concourse.tile as tile
from concourse import mybir
from concourse.bass_utils import run_bass_kernel_spmd

RHO = 1.2929
C_SOUND = 343.37
N_SUB = 128
N_CORES = 8
D0 = 32.0

F32 = mybir.dt.float32
MULT = mybir.AluOpType.mult
ADD = mybir.AluOpType.add
SUB = mybir.AluOpType.subtract
IDENT = mybir.ActivationFunctionType.Identity
COPY = mybir.ActivationFunctionType.Copy
SQUARE = mybir.ActivationFunctionType.Square
SQRT = mybir.ActivationFunctionType.Sqrt


def _sel(tsb, part, base_entry, entry_step, n, m, odd):
    """Strided selection AP over a packed [P, 4*n] plane tile.

    Pattern: entries (e, e, e+s, e+s) x (left|right of each adjacent pair).
    dims: [[entry_step*n, 2], [0, 2], [2, m]] starting at base_entry*n (+1 if odd).
    """
    off = base_entry * n + (1 if odd else 0)
    return bass.AP(tsb, off, [part, [entry_step * n, 2], [0, 2], [2, m]])


def _rsel(tsb, part, base_entry, entry_step, n, m, odd):
    """Right-operand pattern: entries (e, e+s, e, e+s)."""
    off = base_entry * n + (1 if odd else 0)
    return bass.AP(tsb, off, [part, [0, 2], [entry_step * n, 2], [2, m]])


def _outv(tsb, part, m):
    """Contiguous [P, 2, 2, m] view of a packed [P, 4*m] tile."""
    return bass.AP(tsb, 0, [part, [2 * m, 2], [m, 2], [1, m]])


def _emit_body(nc, tc, pool, P, xd, outd):
    """Emit one full evaluation: DMA in -> compute -> DMA out.

    Unified complex plane tile [re(4n) | im(4n)]; per tree level 8 plain
    mults (Vector/GPSIMD split) write a term-interleaved tile and ONE
    tensor_reduce(add, innermost-4) produces both re and im of the next
    level; the real-part negations come from an ACT-built negated-imag half.
    Prep chain runs on Vector in 2x tensor_scalar mode; the radiation load
    ZL is folded into the last slice matrix (E = [[ZL,0],[1,0]]).
    """
    N = N_SUB

    def T(w, tag):
        return pool.tile([P, w], F32, name=tag, tag=tag)

    V, G, S = nc.vector, nc.gpsimd, nc.scalar

    # prefetch the sqrt_and_friends activation table before the input arrives
    warm = T(1, "warm")
    S.activation(warm[:], nc.const_aps.aps[(F32, 1.0)][:P], SQRT)

    x_sb = T(4 + N, "x")
    nc.sync.dma_start(out=x_sb[:, 0:4], in_=xd.ap()[:, 0:4])
    nc.sync.dma_start(out=x_sb[:, 4:4 + N], in_=xd.ap()[:, 4:4 + N])
    f = x_sb[:, 0:1]
    sqf = x_sb[:, 1:2]
    ln = x_sb[:, 2:3]
    d1 = x_sb[:, 3:4]
    tg = x_sb[:, 4:4 + N]

    # --- prep: [P,1] scalars on ACT, [P,N] grids on Vector (2x ts mode) ---
    dL = T(1, "dL")
    S.activation(dL[:], ln, COPY, scale=10.0 / 1000.0 / N_SUB)
    y = T(1, "y")
    V.scalar_tensor_tensor(y[:], f, 2.0 * math.pi / C_SOUND, dL[:], MULT, MULT)
    s_ = T(1, "s_")
    V.scalar_tensor_tensor(s_[:], sqf, 3e-5, dL[:], MULT, MULT)
    dd = T(1, "dd")
    S.activation(dd[:], d1, IDENT, scale=1.0 / 2000.0, bias=-D0 / 2000.0)
    r = T(N, "r")
    V.tensor_scalar(r[:], tg, dd[:], D0 / 2000.0, MULT, ADD)
    rinv = T(N, "rinv")
    V.reciprocal(rinv[:], r[:])
    xg = T(N, "xg")
    V.tensor_scalar(xg[:], rinv[:], s_[:], None, MULT)
    x2 = T(N, "x2")
    V.tensor_mul(x2[:], xg[:], xg[:])
    chx = T(N, "chx")
    V.tensor_scalar(chx[:], x2[:], 0.5, 1.0, MULT, ADD)
    w6 = T(N, "w6")
    V.tensor_scalar(w6[:], x2[:], 1.0 / 6.0, 1.0, MULT, ADD)
    shx = T(N, "shx")
    V.tensor_mul(shx[:], xg[:], w6[:])
    y2 = T(1, "y2")
    S.activation(y2[:], y[:], SQUARE)
    cyh = T(1, "cyh")
    S.activation(cyh[:], y2[:], IDENT, scale=1.0 / 24.0, bias=-0.5)
    cosy = T(1, "cosy")
    S.activation(cosy[:], cyh[:], IDENT, scale=y2[:], bias=1.0)
    syh = T(1, "syh")
    S.activation(syh[:], y2[:], IDENT, scale=1.0 / 120.0, bias=-1.0 / 6.0)
    syw = T(1, "syw")
    S.activation(syw[:], syh[:], IDENT, scale=y2[:], bias=1.0)
    siny = T(1, "siny")
    S.activation(siny[:], syw[:], COPY, scale=y[:])
    nsiny = T(1, "nsiny")
    S.activation(nsiny[:], siny[:], COPY, scale=-1.0)  # -siny
    z0 = T(N, "z0")
    V.scalar_tensor_tensor(z0[:], rinv[:], RHO * C_SOUND / math.pi, rinv[:], MULT, MULT)
    z0i = T(N, "z0i")
    V.scalar_tensor_tensor(z0i[:], r[:], math.pi / (RHO * C_SOUND), r[:], MULT, MULT)
    shc = T(N, "shc")
    V.tensor_scalar(shc[:], shx[:], cosy[:], None, MULT)
    chs = T(N, "chs")
    V.tensor_scalar(chs[:], chx[:], siny[:], None, MULT)

    # radiation impedance ZL [P,1]
    r_end = T(1, "r_end")
    S.activation(r_end[:], d1, COPY, scale=1.0 / 2000.0)
    rinv_e = T(1, "rinv_e")
    V.reciprocal(rinv_e[:], r_end[:])
    kr = T(1, "kr")
    V.scalar_tensor_tensor(kr[:], f, 2.0 * math.pi / C_SOUND, r_end[:], MULT, MULT)
    z0e = T(1, "z0e")
    V.scalar_tensor_tensor(z0e[:], rinv_e[:], RHO * C_SOUND / math.pi, rinv_e[:], MULT, MULT)
    kr2 = T(1, "kr2")
    S.activation(kr2[:], kr[:], SQUARE)
    zlre = T(1, "zlre")
    V.scalar_tensor_tensor(zlre[:], kr2[:], 0.25, z0e[:], MULT, MULT)
    zlim = T(1, "zlim")
    V.scalar_tensor_tensor(zlim[:], kr[:], 0.61, z0e[:], MULT, MULT)

    # --- level-0 planes: unified [P, re(A,B,C,D) | im(A,B,C,D)] ---
    # layout: re entries at 0,N,2N,3N ; im at 4N..7N (D = A at level 0).
    # Slices 0..126 come from the bulk builds; slice 127 (the E-fold column,
    # T'127 = T127 @ [[ZL,0],[1,0]]) is computed straight from prep values so
    # the fold runs CONCURRENTLY with the bulk plane builds.
    M = N - 1
    lc = N - 1
    pc = T(8 * N, "pc0")
    S.activation(pc[:, 0:M], chx[:, 0:M], COPY, scale=cosy[:])           # A_re
    S.activation(pc[:, 4 * N:4 * N + M], shx[:, 0:M], COPY, scale=siny[:])   # A_im
    V.tensor_mul(pc[:, N:N + M], z0[:, 0:M], shc[:, 0:M])                # B_re
    V.tensor_mul(pc[:, 5 * N:5 * N + M], z0[:, 0:M], chs[:, 0:M])        # B_im
    G.tensor_mul(pc[:, 2 * N:2 * N + M], z0i[:, 0:M], shc[:, 0:M])       # C_re
    G.tensor_mul(pc[:, 6 * N:6 * N + M], z0i[:, 0:M], chs[:, 0:M])       # C_im
    S.activation(pc[:, 3 * N:3 * N + M], chx[:, 0:M], COPY, scale=cosy[:])   # D_re
    S.activation(pc[:, 7 * N:7 * N + M], shx[:, 0:M], COPY, scale=siny[:])   # D_im

    # negated imag half for level-1 real-part products (slices 0..126)
    ng = T(4 * N, "ng0")
    S.activation(ng[:, 0:M], shx[:, 0:M], COPY, scale=nsiny[:])          # -A_im
    S.activation(ng[:, N:N + M], pc[:, 5 * N:5 * N + M], COPY, scale=-1.0)   # -B_im
    S.activation(ng[:, 2 * N:2 * N + M], pc[:, 6 * N:6 * N + M], COPY, scale=-1.0)  # -C_im
    S.activation(ng[:, 3 * N:3 * N + M], shx[:, 0:M], COPY, scale=nsiny[:])  # -D_im

    # folded column 127, from prep values only (parallel with bulk builds):
    # T127 entries, then A' = A*ZL + B ; C' = C*ZL + A ; B' = D' = 0
    ch7 = chx[:, lc:lc + 1]
    sh7 = shx[:, lc:lc + 1]
    ar0 = T(1, "ar0")
    V.tensor_scalar(ar0[:], ch7, cosy[:], None, MULT)        # A127 re
    ai0 = T(1, "ai0")
    V.tensor_scalar(ai0[:], sh7, siny[:], None, MULT)        # A127 im
    sc0 = T(1, "sc0")
    V.tensor_scalar(sc0[:], sh7, cosy[:], None, MULT)        # sh*cosy
    ci0 = T(1, "ci0")
    V.tensor_scalar(ci0[:], ch7, siny[:], None, MULT)        # ch*siny
    br0 = T(1, "br0")
    G.tensor_mul(br0[:], z0[:, lc:lc + 1], sc0[:])           # B127 re
    bi0 = T(1, "bi0")
    G.tensor_mul(bi0[:], z0[:, lc:lc + 1], ci0[:])           # B127 im
    cr0 = T(1, "cr0")
    G.tensor_mul(cr0[:], z0i[:, lc:lc + 1], sc0[:])          # C127 re
    cib = T(1, "cib")
    G.tensor_mul(cib[:], z0i[:, lc:lc + 1], ci0[:])          # C127 im
    e1 = T(1, "e1")
    V.tensor_scalar(e1[:], ar0[:], zlre[:], br0[:], MULT, ADD)   # Are*ZLre + Bre
    e2 = T(1, "e2")
    V.tensor_scalar(e2[:], ai0[:], zlim[:], None, MULT)          # Aim*ZLim
    e3 = T(1, "e3")
    V.tensor_scalar(e3[:], ar0[:], zlim[:], bi0[:], MULT, ADD)   # Are*ZLim + Bim
    e4 = T(1, "e4")
    V.tensor_scalar(e4[:], ai0[:], zlre[:], None, MULT)          # Aim*ZLre
    g1 = T(1, "g1")
    V.tensor_scalar(g1[:], cr0[:], zlre[:], ar0[:], MULT, ADD)   # Cre*ZLre + Dre(=Are)
    g2 = T(1, "g2")
    V.tensor_scalar(g2[:], cib[:], zlim[:], None, MULT)
    g3 = T(1, "g3")
    V.tensor_scalar(g3[:], cr0[:], zlim[:], ai0[:], MULT, ADD)
    g4 = T(1, "g4")
    V.tensor_scalar(g4[:], cib[:], zlre[:], None, MULT)
    G.tensor_sub(pc[:, lc:lc + 1], e1[:], e2[:])                 # A'127 re
    G.tensor_add(pc[:, 4 * N + lc:4 * N + lc + 1], e3[:], e4[:])  # A'127 im
    G.tensor_sub(pc[:, 2 * N + lc:2 * N + lc + 1], g1[:], g2[:])  # C'127 re
    G.tensor_add(pc[:, 6 * N + lc:6 * N + lc + 1], g3[:], g4[:])  # C'127 im
    # B'127 = D'127 = 0 and ng column 127 (from const-0; cols never written
    # by the bulk builds, so fill fresh rather than in-place scaling)
    zero_ap = nc.const_aps.aps[(F32, 0.0)][:P]
    S.activation(pc[:, N + lc:N + lc + 1], zero_ap, COPY)         # B'127 re
    S.activation(pc[:, 5 * N + lc:5 * N + lc + 1], zero_ap, COPY)  # B'127 im
    S.activation(pc[:, 3 * N + lc:3 * N + lc + 1], zero_ap, COPY)  # D'127 re
    S.activation(pc[:, 7 * N + lc:7 * N + lc + 1], zero_ap, COPY)  # D'127 im
    S.activation(ng[:, lc:lc + 1], pc[:, 4 * N + lc:4 * N + lc + 1], COPY, scale=-1.0)
    S.activation(ng[:, N + lc:N + lc + 1], zero_ap, COPY)
    S.activation(ng[:, 2 * N + lc:2 * N + lc + 1], pc[:, 6 * N + lc:6 * N + lc + 1], COPY, scale=-1.0)
    S.activation(ng[:, 3 * N + lc:3 * N + lc + 1], zero_ap, COPY)

    # --- binary tree: per level 8 mults + 1 fused reduce ---
    n = N
    lvl = 0
    im_off = 4 * N  # offset of the imag half in the current plane tile
    ng_t = ng
    while n > 1:
        m = n // 2
        lvl += 1
        h = pc[:].tensor
        pd = [pc[:].ap[0][0], P]
        hn = ng_t[:].tensor
        pdn = [ng_t[:].ap[0][0], P]

        l1r = bass.AP(h, 0, [pd, [2 * n, 2], [0, 2], [2, m]])
        l1i = bass.AP(h, im_off, [pd, [2 * n, 2], [0, 2], [2, m]])
        l1n = bass.AP(hn, 0, [pdn, [2 * n, 2], [0, 2], [2, m]])
        r1r = bass.AP(h, 1, [pd, [0, 2], [n, 2], [2, m]])
        r1i = bass.AP(h, im_off + 1, [pd, [0, 2], [n, 2], [2, m]])
        l2r = bass.AP(h, n, [pd, [2 * n, 2], [0, 2], [2, m]])
        l2i = bass.AP(h, im_off + n, [pd, [2 * n, 2], [0, 2], [2, m]])
        l2n = bass.AP(hn, n, [pdn, [2 * n, 2], [0, 2], [2, m]])
        r2r = bass.AP(h, 2 * n + 1, [pd, [0, 2], [n, 2], [2, m]])
        r2i = bass.AP(h, im_off + 2 * n + 1, [pd, [0, 2], [n, 2], [2, m]])

        # term-interleaved products: re terms at c=0, im at c=1
        # element (c, e, p, t) at c*16m + 4*(e*m+p) + t
        u = T(32 * m, f"u{lvl}")
        uh = u[:].tensor
        upd = [u[:].ap[0][0], P]

        def tm(c, t):
            return bass.AP(uh, c * 16 * m + t, [upd, [8 * m, 2], [4 * m, 2], [4, m]])

        # real part: t0=Lre1*Rre1 t1=Lre2*Rre2 t2=(-Lim1)*Rim1 t3=(-Lim2)*Rim2
        V.tensor_tensor(tm(0, 0), l1r, r1r, MULT)
        V.tensor_tensor(tm(0, 1), l2r, r2r, MULT)
        # imag part: Lre*Rim + Lim*Rre; G long pole at small levels -> shift
        # one imag mult to Vector there
        (V if n <= 16 else G).tensor_tensor(tm(1, 0), l1r, r1i, MULT)
        G.tensor_tensor(tm(1, 1), l2r, r2i, MULT)
        G.tensor_tensor(tm(1, 2), l1i, r1r, MULT)
        G.tensor_tensor(tm(1, 3), l2i, r2r, MULT)
        # negim-dependent last (off Vector at big levels so the reduces
        # don't wait on the ACT-negate hop)
        (G if n >= 64 else V).tensor_tensor(tm(0, 2), l1n, r1i, MULT)
        G.tensor_tensor(tm(0, 3), l2n, r2i, MULT)

        q = T(8 * m, f"pc{lvl}")
        rin_r = bass.AP(uh, 0, [upd, [4, 4 * m], [1, 4]])
        rin_i = bass.AP(uh, 16 * m, [upd, [4, 4 * m], [1, 4]])
        V.tensor_reduce(q[:, 0:4 * m], rin_r, mybir.AxisListType.X, ADD)
        V.tensor_reduce(q[:, 4 * m:8 * m], rin_i, mybir.AxisListType.X, ADD)

        if m > 1:
            ngn = T(4 * m, f"ng{lvl}")
            S.activation(ngn[:], q[:, 4 * m:8 * m], COPY, scale=-1.0)
            ng_t = ngn
        pc = q
        im_off = 4 * m
        n = m

    # --- final: num = A (entries 0re / 4im), den = C (2re / 6im) ---
    are, aim = pc[:, 0:1], pc[:, 4:5]
    cre, cim = pc[:, 2:3], pc[:, 6:7]
    n2a = T(1, "n2a")
    S.activation(n2a[:], are, SQUARE)
    n2b = T(1, "n2b")
    S.activation(n2b[:], aim, SQUARE)
    n2 = T(1, "n2")
    V.tensor_add(n2[:], n2a[:], n2b[:])
    d2a = T(1, "d2a")
    S.activation(d2a[:], cre, SQUARE)
    d2b = T(1, "d2b")
    S.activation(d2b[:], cim, SQUARE)
    d2 = T(1, "d2")
    G.tensor_add(d2[:], d2a[:], d2b[:])
    d2r = T(1, "d2r")
    V.reciprocal(d2r[:], d2[:])
    rat = T(1, "rat")
    V.tensor_mul(rat[:], n2[:], d2r[:])
    res = T(1, "res")
    S.activation(res[:], rat[:], SQRT)

    nc.sync.dma_start(out=outd.ap(), in_=res[:])


def build_program(fpc, loop_iters=None):
    """Build the SPMD Bass program; every core runs it on its own 47 freqs.

    loop_iters: if set, wrap the body in a hardware For_i loop (used only by
    the timing harness to amortize dispatch overhead)."""
    nc = bacc.Bacc("TRN2", target_bir_lowering=False, debug=False)
    P = fpc
    N = N_SUB

    # activation-bias constants beyond the built-in 0.0/1.0
    for cv in (-D0 / 2000.0, D0 / 2000.0, -0.5, -1.0 / 6.0):
        th = nc.alloc_sbuf_tensor(f"cst{cv}", [128, 1], F32)
        nc.gpsimd.memset(th.ap(), cv)
        nc.const_aps.aps[(F32, cv)] = th.ap()
    nc.all_engine_barrier()

    xd = nc.dram_tensor("x", [P, 4 + N], F32, kind="ExternalInput")
    outd = nc.dram_tensor("out", [P, 1], F32, kind="ExternalOutput")

    with tile.TileContext(nc) as tc, ExitStack() as ctx:
        pool = ctx.enter_context(tc.tile_pool(name="p", bufs=1))
        if loop_iters is None:
            _emit_body(nc, tc, pool, P, xd, outd)
        else:
            with tc.For_i(0, loop_iters, 1):
                _emit_body(nc, tc, pool, P, xd, outd)

    nc.compile()
    return nc


_PROGRAM_CACHE = {}


def _get_program(fpc):
    if fpc not in _PROGRAM_CACHE:
        _PROGRAM_CACHE[fpc] = build_program(fpc)
    return _PROGRAM_CACHE[fpc]


def make_inputs(length, d1, fmin, fmax, fpc):
    """Host-side shard prep: pack [f | length | d1 | t] per core. No math on
    device-owned values beyond replication."""
    F = fmax - fmin
    f_full = np.arange(fmin, fmax, dtype=np.float32)
    f_pad = np.concatenate([f_full, np.full(N_CORES * fpc - F, float(fmin), np.float32)])
    t = ((np.arange(N_SUB, dtype=np.float32) + 0.5) / N_SUB)
    in_maps = []
    for c in range(N_CORES):
        X = np.empty((fpc, 4 + N_SUB), dtype=np.float32)
        X[:, 0] = f_pad[c * fpc:(c + 1) * fpc]
        X[:, 1] = np.sqrt(f_pad[c * fpc:(c + 1) * fpc])
        X[:, 2] = np.float32(length[0])
        X[:, 3] = np.float32(d1[0])
        X[:, 4:] = t[None, :]
        in_maps.append({"x": X})
    return in_maps


def kernel(length, d1, fmin, fmax):
    length = np.asarray(length, dtype=np.float32)
    d1 = np.asarray(d1, dtype=np.float32)
    fmin = int(fmin)
    fmax = int(fmax)
    F = fmax - fmin
    fpc = (F + N_CORES - 1) // N_CORES
    nc = _get_program(fpc)
    in_maps = make_inputs(length, d1, fmin, fmax, fpc)
    res = run_bass_kernel_spmd(nc, in_maps, list(range(N_CORES)))
    outs = [res.results[c]["out"].reshape(-1) for c in range(N_CORES)]
    return np.concatenate(outs)[:F].astype(np.float32)



# revision 6
# speedup vs baseline: 1.2583x; 1.2583x over previous
"""Trainium2 Bass kernel for the didgeridoo (conical bore) input-impedance model.

Math: the reference chains 128 per-slice lossy transmission-line matrices
T_n = exp(X_n), X_n = gamma_n*[[0, z0_n],[1/z0_n, 0]] per frequency, then
Ze = (A*ZL+B)/(C*ZL+D) against the radiation load, output |Ze|.

This kernel replaces the 7-level 128-leaf matrix-product tree with a
2nd-order Magnus (group-exponential) method: slices are grouped 8 at a time;
within a group Omega = sum X_n + 0.5*sum_{i<j}[X_i,X_j] = [[d,b],[c,-d]] and
exp(Omega) = [[C+dS, bS],[cS, C-dS]] with C = cosh(sqrt(w)), S = sinh/sqrt,
w = b*c (d^2 dropped), both EVEN power series in w — no sqrt needed. b, c
reduce to per-group geometry sums (radix-8 tensor_reduce of rinv^3, rinv^2,
r, r^2 with per-frequency scalar prefactors); the commutator term d uses a
linearization of ln z0 over the group (W = 168*delta*mean(rinv), verified to
2.3e-3 max rel err vs the reference, tolerance 2e-2). Only a 4-level binary
tree over 16 group matrices remains; per level 8 strided tensor-multiplies
(negations folded in via scalar_tensor_tensor) + ONE fused radix-4 reduce.
Complex arithmetic is packed re|im on the free axis so most series/assembly
steps are single instructions; work is split across Vector/GPSIMD/Act.

Sharding (per the hint): frequencies sharded 8 ways (47 per core, padded);
each core holds its frequencies on the SBUF partition axis.
"""
import math
from contextlib import ExitStack

import numpy as np

import concourse.bass as bass
import concourse.bacc as bacc
import concourse.tile as tile
from concourse import mybir
from concourse.bass_utils import run_bass_kernel_spmd

RHO = 1.2929
C_SOUND = 343.37
N_SUB = 128
N_CORES = 8
D0 = 32.0
K1 = RHO * C_SOUND / math.pi        # z0 = K1 / r^2
K2 = math.pi / (RHO * C_SOUND)

F32 = mybir.dt.float32
MULT = mybir.AluOpType.mult
ADD = mybir.AluOpType.add
SUB = mybir.AluOpType.subtract
COPY = mybir.ActivationFunctionType.Copy
SQRT = mybir.ActivationFunctionType.Sqrt
X_AX = mybir.AxisListType.X


def _emit_body(nc, tc, pool, P, xd, outd):
    """One full evaluation: DMA in -> Magnus groups -> 4-level tree -> DMA out."""
    N = N_SUB
    V, G, S = nc.vector, nc.gpsimd, nc.scalar

    def T(w, tag):
        return pool.tile([P, w], F32, name=tag, tag=tag)

    def ap(tile_, off, dims):
        return bass.AP(tile_[:].tensor, off, [[tile_[:].ap[0][0], P]] + dims)

    # prefetch sqrt activation table before the input lands
    warm = T(1, "warm")
    S.activation(warm[:], nc.const_aps.aps[(F32, 1.0)][:P], SQRT)

    x = T(4 + N, "x")
    nc.sync.dma_start(out=x[:], in_=xd.ap())
    f = x[:, 0:1]
    sqf = x[:, 1:2]
    ln = x[:, 2:3]
    d1 = x[:, 3:4]
    tg = x[:, 4:4 + N]

    # ---- per-partition scalars --------------------------------------------
    dd = T(1, "dd")          # (d1-32)/2000
    V.tensor_scalar(dd[:], d1, 5e-4, -D0 / 2000.0, MULT, ADD)
    dL2 = T(1, "dL2")        # (2*pi/C)*dL per unit f
    V.tensor_scalar(dL2[:], ln, 2.0 * math.pi / C_SOUND / 12800.0, None, MULT)
    dL3 = T(1, "dL3")        # 3e-5*dL
    V.tensor_scalar(dL3[:], ln, 3e-5 / 12800.0, None, MULT)
    y = T(1, "y")            # k*dL
    G.tensor_mul(y[:], f, dL2[:])
    s_ = T(1, "s_")          # 3e-5*sqrt(f)*dL
    G.tensor_mul(s_[:], sqf, dL3[:])
    y_sq = T(1, "y_sq")
    G.tensor_mul(y_sq[:], y[:], y[:])
    s_sq = T(1, "s_sq")
    G.tensor_mul(s_sq[:], s_[:], s_[:])
    sy = T(1, "sy")
    G.tensor_mul(sy[:], s_[:], y[:])
    # b,c prefactors (Act, COPY with per-partition scale)
    s1 = T(1, "s1")
    S.activation(s1[:], s_[:], COPY, scale=K1)
    y1 = T(1, "y1")
    S.activation(y1[:], y[:], COPY, scale=K1)
    s2 = T(1, "s2")
    S.activation(s2[:], s_[:], COPY, scale=K2)
    y2 = T(1, "y2")
    S.activation(y2[:], y[:], COPY, scale=K2)
    # d prefactors: c_dre = -21/128 * dd * y^2 ; c_dim = 5.25/128 * dd * s*y
    ydd = T(1, "ydd")
    G.tensor_mul(ydd[:], y_sq[:], dd[:])
    c_dre = T(1, "c_dre")
    G.tensor_scalar(c_dre[:], ydd[:], -21.0 / 128.0, None, MULT)
    sdd = T(1, "sdd")
    G.tensor_mul(sdd[:], sy[:], dd[:])
    c_dim = T(1, "c_dim")
    G.tensor_scalar(c_dim[:], sdd[:], 5.25 / 128.0, None, MULT)
    # radiation load ZL scalars (K1 folded into the 0.25/0.61 prefactors)
    r_end = T(1, "r_end")
    V.tensor_scalar(r_end[:], d1, 5e-4, None, MULT)
    rei = T(1, "rei")
    V.reciprocal(rei[:], r_end[:])
    z0e = T(1, "z0e")        # rei^2 (unscaled)
    G.tensor_mul(z0e[:], rei[:], rei[:])
    re2 = T(1, "re2")        # (2*pi/C)*r_end
    G.tensor_scalar(re2[:], r_end[:], 2.0 * math.pi / C_SOUND, None, MULT)
    zkr = T(1, "zkr")
    G.tensor_mul(zkr[:], f, re2[:])
    zkr2 = T(1, "zkr2")
    G.tensor_mul(zkr2[:], zkr[:], zkr[:])
    zlre = T(1, "zlre")
    V.scalar_tensor_tensor(zlre[:], zkr2[:], 0.25 * K1, z0e[:], MULT, MULT)
    zlim = T(1, "zlim")
    V.scalar_tensor_tensor(zlim[:], zkr[:], 0.61 * K1, z0e[:], MULT, MULT)

    # ---- slice-geometry stage: Q = [rinv^3 | rinv^2 | r | r^2 | rinv] -----
    Q = T(5 * N, "Q")
    V.tensor_scalar(Q[:, 2 * N:3 * N], tg, dd[:], D0 / 2000.0, MULT, ADD)   # r
    V.reciprocal(Q[:, 4 * N:5 * N], Q[:, 2 * N:3 * N])                      # rinv
    # squares: in = [r (slot2) | rinv (slot4)] stride-2N; out = [r^2(3) | rinv^2(1)]
    V.tensor_tensor(ap(Q, 3 * N, [[-2 * N, 2], [1, N]]),
                    ap(Q, 2 * N, [[2 * N, 2], [1, N]]),
                    ap(Q, 2 * N, [[2 * N, 2], [1, N]]), MULT)
    V.tensor_mul(Q[:, 0:N], Q[:, N:2 * N], Q[:, 4 * N:5 * N])               # rinv^3

    # ---- group sums (radix-8): R = [R1|R2|R3|R4], R5 ----------------------
    R = T(64, "R")
    V.tensor_reduce(R[:], ap(Q, 0, [[8, 64], [1, 8]]), X_AX, ADD)
    R5 = T(16, "R5")
    V.tensor_reduce(R5[:], ap(Q, 4 * N, [[8, 16], [1, 8]]), X_AX, ADD)

    # ---- w = b*c (d^2 dropped), packed [w_re | w_im] ----------------------
    # w_re = s^2*(R1*R3) - y^2*(R2*R4) ; w_im = s*y*(R1*R4 + R2*R3)
    P1 = T(32, "P1")
    V.tensor_tensor(P1[:], R[:, 0:32], R[:, 32:64], MULT)                  # [R1R3|R2R4]
    P2 = T(32, "P2")
    V.tensor_tensor(P2[:], R[:, 0:32], ap(R, 48, [[-16, 2], [1, 16]]), MULT)  # [R1R4|R2R3]
    q_ = T(16, "q_")
    V.tensor_scalar(q_[:], P1[:, 16:32], y_sq[:], None, MULT)
    WP = T(32, "WP")
    V.scalar_tensor_tensor(WP[:, 0:16], P1[:, 0:16], s_sq[:], q_[:], MULT, SUB)
    p2s = T(16, "p2s")
    G.tensor_add(p2s[:], P2[:, 0:16], P2[:, 16:32])
    G.tensor_scalar(WP[:, 16:32], p2s[:], sy[:], None, MULT)

    # ---- powers w^2, w^3 (packed complex) ---------------------------------
    SQt = T(32, "SQt")
    V.tensor_tensor(SQt[:], WP[:], WP[:], MULT)                            # [wre^2|wim^2]
    W2 = T(32, "W2")
    V.tensor_tensor(W2[:, 0:16], SQt[:, 0:16], SQt[:, 16:32], SUB)
    V.scalar_tensor_tensor(W2[:, 16:32], WP[:, 0:16], 2.0, WP[:, 16:32], MULT, MULT)
    Q3 = T(32, "Q3")
    V.tensor_tensor(Q3[:], W2[:], WP[:], MULT)                             # [w2re*wre|w2im*wim]
    Q4 = T(32, "Q4")
    V.tensor_tensor(Q4[:], W2[:], ap(WP, 16, [[-16, 2], [1, 16]]), MULT)   # [w2re*wim|w2im*wre]
    W3 = T(32, "W3")
    V.tensor_tensor(W3[:, 0:16], Q3[:, 0:16], Q3[:, 16:32], SUB)
    G.tensor_add(W3[:, 16:32], Q4[:, 0:16], Q4[:, 16:32])

    # ---- series: C = 1+(12w+w2+w3/30)/24 ; S = 1+(840w+42w2+w3)/5040 ------
    c1_ = T(32, "c1_")
    V.scalar_tensor_tensor(c1_[:], W3[:], 1.0 / 30.0, W2[:], MULT, ADD)
    c2_ = T(32, "c2_")
    V.scalar_tensor_tensor(c2_[:], WP[:], 12.0, c1_[:], MULT, ADD)
    CT = T(32, "CT")
    V.tensor_scalar(CT[:, 0:16], c2_[:, 0:16], 1.0 / 24.0, 1.0, MULT, ADD)
    V.tensor_scalar(CT[:, 16:32], c2_[:, 16:32], 1.0 / 24.0, None, MULT)
    w2s = T(32, "w2s")
    G.tensor_scalar(w2s[:], W2[:], 42.0, None, MULT)
    sg1 = T(32, "sg1")
    G.tensor_add(sg1[:], w2s[:], W3[:])
    wps = T(32, "wps")
    G.tensor_scalar(wps[:], WP[:], 840.0, None, MULT)
    sg2 = T(32, "sg2")
    G.tensor_add(sg2[:], wps[:], sg1[:])
    S4 = T(64, "S4")                       # [Sre|Sim|Sim|Sre] via doubled writes
    G.tensor_scalar(ap(S4, 0, [[48, 2], [1, 16]]),
                    ap(sg2, 0, [[0, 2], [1, 16]]), 1.0 / 5040.0, 1.0, MULT, ADD)
    G.tensor_scalar(ap(S4, 16, [[16, 2], [1, 16]]),
                    ap(sg2, 16, [[0, 2], [1, 16]]), 1.0 / 5040.0, None, MULT)

    # ---- left operands LT = [b_re|b_im|c_re|c_im|d_re|d_im] ---------------
    LT = T(96, "LT")
    S.activation(LT[:, 0:16], R[:, 0:16], COPY, scale=s1[:])
    S.activation(LT[:, 16:32], R[:, 16:32], COPY, scale=y1[:])
    S.activation(LT[:, 32:48], R[:, 32:48], COPY, scale=s2[:])
    S.activation(LT[:, 48:64], R[:, 48:64], COPY, scale=y2[:])
    V.tensor_scalar(LT[:, 64:80], R5[:], c_dre[:], None, MULT)
    R5sq = T(16, "R5sq")
    G.tensor_mul(R5sq[:], R5[:], R5[:])
    G.tensor_scalar(LT[:, 80:96], R5sq[:], c_dim[:], None, MULT)

    # ---- entry assembly into V0 [P,128] = re(A,B,C,D)|im(A,B,C,D) ---------
    V0 = T(128, "V0")
    PP1 = T(96, "PP1")
    V.tensor_tensor(PP1[:], LT[:], ap(S4, 0, [[0, 3], [1, 32]]), MULT)
    PP2 = T(96, "PP2")
    V.tensor_tensor(PP2[:], LT[:], ap(S4, 32, [[0, 3], [1, 32]]), MULT)
    # X_re: [bS_re->B_re(16) | cS_re->C_re(32) | dS_re->scratch(48)]
    V.tensor_tensor(ap(V0, 16, [[16, 3], [1, 16]]),
                    ap(PP1, 0, [[32, 3], [1, 16]]),
                    ap(PP1, 16, [[32, 3], [1, 16]]), SUB)
    G.tensor_tensor(ap(V0, 80, [[16, 3], [1, 16]]),
                    ap(PP2, 0, [[32, 3], [1, 16]]),
                    ap(PP2, 16, [[32, 3], [1, 16]]), ADD)
    V.tensor_add(V0[:, 0:16], CT[:, 0:16], V0[:, 48:64])      # A_re
    V.tensor_sub(V0[:, 48:64], CT[:, 0:16], V0[:, 48:64])     # D_re (in place)
    G.tensor_add(V0[:, 64:80], CT[:, 16:32], V0[:, 112:128])  # A_im
    G.tensor_sub(V0[:, 112:128], CT[:, 16:32], V0[:, 112:128])  # D_im
    NX = T(64, "NX0")                                  # negated im half
    S.activation(NX[:], V0[:, 64:128], COPY, scale=-1.0)

    # ---- 4-level tree over 16 group matrices ------------------------------
    # per level: 8 strided products (negated-im tile supplies the minus
    # terms), one fused radix-4 reduce -> next level, one negated radix-4
    # reduce of the im terms -> next level's negated-im tile.
    X = V0
    n = 16
    lvl = 0
    while n > 1:
        m = n // 2
        lvl += 1
        io = 4 * n
        U = T(32 * m, f"U{lvl}")
        Xn = T(8 * m, f"X{lvl}")

        def uo(c, k, s_i):
            return ap(U, c * 16 * m + 2 * k + s_i, [[8 * m, 2], [4 * m, 2], [4, m]])

        def li(k, imag):
            return ap(X, k * n + (io if imag else 0), [[2 * n, 2], [0, 2], [2, m]])

        def lin(k):
            return ap(NX, k * n, [[2 * n, 2], [0, 2], [2, m]])

        def ri(k, imag):
            return ap(X, 2 * k * n + 1 + (io if imag else 0), [[0, 2], [n, 2], [2, m]])

        # re: + Lre*Rre (k=0,1), - Lim*Rim via negated-im tile
        V.tensor_tensor(uo(0, 0, 0), li(0, 0), ri(0, 0), MULT)
        V.tensor_tensor(uo(0, 1, 0), li(1, 0), ri(1, 0), MULT)
        V.tensor_tensor(uo(0, 0, 1), lin(0), ri(0, 1), MULT)
        G.tensor_tensor(uo(0, 1, 1), lin(1), ri(1, 1), MULT)
        # im: Lre*Rim + Lim*Rre
        V.tensor_tensor(uo(1, 0, 0), li(0, 0), ri(0, 1), MULT)
        G.tensor_tensor(uo(1, 1, 0), li(1, 0), ri(1, 1), MULT)
        V.tensor_tensor(uo(1, 0, 1), li(0, 1), ri(0, 0), MULT)
        G.tensor_tensor(uo(1, 1, 1), li(1, 1), ri(1, 0), MULT)
        V.tensor_reduce(Xn[:], ap(U, 0, [[4, 8 * m], [1, 4]]), X_AX, ADD)
        if m > 1:
            NXn = T(4 * m, f"NX{lvl}")
            V.tensor_reduce(NXn[:], ap(U, 16 * m, [[4, 4 * m], [1, 4]]),
                            X_AX, ADD, negate=True)
            NX = NXn
        X = Xn
        n = m

    # ---- tail: Ze = (A*ZL+B)/(C*ZL+D), |Ze| -------------------------------
    # X [P,8] = [Ar|Br|Cr|Dr|Ai|Bi|Ci|Di]
    TT = T(8, "TT")
    acv = ap(X, 0, [[2, 4]])                       # [Ar,Cr,Ai,Ci]
    V.tensor_scalar(TT[:, 0:4], acv, zlre[:], None, MULT)
    G.tensor_scalar(TT[:, 4:8], acv, zlim[:], None, MULT)
    SS = T(4, "SS")
    V.tensor_sub(SS[:, 0:2], TT[:, 0:2], TT[:, 6:8])   # [ArZr-AiZi, CrZr-CiZi]
    G.tensor_add(SS[:, 2:4], TT[:, 4:6], TT[:, 2:4])   # [ArZi+AiZr, CrZi+CiZr]
    nd = T(4, "nd")
    V.tensor_tensor(nd[:], SS[:], ap(X, 1, [[2, 4]]), ADD)  # +[Br,Dr,Bi,Di]
    sq = T(4, "sq")
    V.tensor_mul(sq[:], nd[:], nd[:])
    n2d2 = T(2, "n2d2")
    V.tensor_add(n2d2[:], sq[:, 0:2], sq[:, 2:4])
    rd = T(1, "rd")
    V.reciprocal(rd[:], n2d2[:, 1:2])
    rat = T(1, "rat")
    V.tensor_mul(rat[:], n2d2[:, 0:1], rd[:])
    res = T(1, "res")
    S.activation(res[:], rat[:], SQRT)
    nc.sync.dma_start(out=outd.ap(), in_=res[:])


def build_program(fpc, loop_iters=None):
    """Build the SPMD Bass program; every core runs it on its own 47 freqs."""
    nc = bacc.Bacc("TRN2", target_bir_lowering=False, debug=False)
    P = fpc
    N = N_SUB

    xd = nc.dram_tensor("x", [P, 4 + N], F32, kind="ExternalInput")
    outd = nc.dram_tensor("out", [P, 1], F32, kind="ExternalOutput")

    with tile.TileContext(nc) as tc, ExitStack() as ctx:
        pool = ctx.enter_context(tc.tile_pool(name="p", bufs=1))
        if loop_iters is None:
            _emit_body(nc, tc, pool, P, xd, outd)
        else:
            with tc.For_i(0, loop_iters, 1):
                _emit_body(nc, tc, pool, P, xd, outd)

    nc.compile()
    return nc


_PROGRAM_CACHE = {}


def _get_program(fpc):
    if fpc not in _PROGRAM_CACHE:
        _PROGRAM_CACHE[fpc] = build_program(fpc)
    return _PROGRAM_CACHE[fpc]


def make_inputs(length, d1, fmin, fmax, fpc):
    """Host-side shard prep: pack [f | sqrt(f) | length | d1 | t] per core."""
    F = fmax - fmin
    f_full = np.arange(fmin, fmax, dtype=np.float32)
    f_pad = np.concatenate([f_full, np.full(N_CORES * fpc - F, float(fmin), np.float32)])
    t = ((np.arange(N_SUB, dtype=np.float32) + 0.5) / N_SUB)
    in_maps = []
    for c in range(N_CORES):
        X = np.empty((fpc, 4 + N_SUB), dtype=np.float32)
        X[:, 0] = f_pad[c * fpc:(c + 1) * fpc]
        X[:, 1] = np.sqrt(f_pad[c * fpc:(c + 1) * fpc])
        X[:, 2] = np.float32(length[0])
        X[:, 3] = np.float32(d1[0])
        X[:, 4:] = t[None, :]
        in_maps.append({"x": X})
    return in_maps


def kernel(length, d1, fmin, fmax):
    length = np.asarray(length, dtype=np.float32)
    d1 = np.asarray(d1, dtype=np.float32)
    fmin = int(fmin)
    fmax = int(fmax)
    F = fmax - fmin
    fpc = (F + N_CORES - 1) // N_CORES
    nc = _get_program(fpc)
    in_maps = make_inputs(length, d1, fmin, fmax, fpc)
    res = run_bass_kernel_spmd(nc, in_maps, list(range(N_CORES)))
    outs = [res.results[c]["out"].reshape(-1) for c in range(N_CORES)]
    return np.concatenate(outs)[:F].astype(np.float32)


# revision 19
# speedup vs baseline: 2.0469x; 1.6266x over previous
"""Trainium2 Bass kernel for the didgeridoo (conical bore) input-impedance model.

Math: the reference chains 128 per-slice lossy transmission-line matrices
T_n = exp(X_n), X_n = gamma_n*[[0, z0_n],[1/z0_n, 0]] per frequency, then
Ze = (A*ZL+B)/(C*ZL+D) against the radiation load, output |Ze|.

Algorithm: 2nd-order Magnus (group-exponential) over groups of 8 slices:
Omega = sum X + 0.5*sum_{i<j}[X_i,X_j] = [[d,b],[c,-d]], exp(Omega) =
[[C+dS, bS],[cS, C-dS]] with C = cosh(sqrt(w)), S = sinh(sqrt(w))/sqrt(w),
w = b*c — both entire EVEN series in w (no sqrt). b, c are per-group
geometry sums computed by ONE radix-8 tensor_reduce over per-frequency
PRE-SCALED slice tensors, so the reduce emits b_re|b_im|c_re|c_im directly.
The commutator d uses the linearization W = 168*delta*mean(rinv) (2.3e-3
max rel err vs reference; tolerance 2e-2). A 3-level binary tree combines
16 group matrices into 2; the radiation load is folded into the final
2-matrix combine, and |.|^2 ratios use fused tensor_tensor_reduce.

Schedule: per-instruction dependency latency (~284 ns same-engine, ~+150
cross-engine, measured) dominates, so the kernel minimizes critical-path
DEPTH: the main chain stays on Vector; GPSIMD/Act run scalar prep, series
coefficients and negated twins in parallel. Packed re|im complex layout
makes most steps single instructions.

Sharding (per the hint): frequencies sharded 8 ways (47 per core, padded),
frequencies on the SBUF partition axis.
"""
import math
from contextlib import ExitStack

import numpy as np

import concourse.bass as bass
import concourse.bacc as bacc
import concourse.tile as tile
from concourse import mybir
from concourse.bass_utils import run_bass_kernel_spmd

RHO = 1.2929
C_SOUND = 343.37
N_SUB = 128
N_CORES = 8
D0 = 32.0
K1 = RHO * C_SOUND / math.pi        # z0 = K1 / r^2
K2 = math.pi / (RHO * C_SOUND)

F32 = mybir.dt.float32
MULT = mybir.AluOpType.mult
ADD = mybir.AluOpType.add
SUB = mybir.AluOpType.subtract
DIV = mybir.AluOpType.divide
COPY = mybir.ActivationFunctionType.Copy
SQRT = mybir.ActivationFunctionType.Sqrt
X_AX = mybir.AxisListType.X


def _emit_body(nc, tc, pool, P, xd, outd, sfx=""):
    N = N_SUB
    V, G, S = nc.vector, nc.gpsimd, nc.scalar

    def T(w, tag):
        tag = tag + sfx
        return pool.tile([P, w], F32, name=tag, tag=tag)

    def ap(tile_, off, dims):
        return bass.AP(tile_[:].tensor, off, [[tile_[:].ap[0][0], P]] + dims)

    warm = T(1, "warm")
    S.activation(warm[:], nc.const_aps.aps[(F32, 1.0)][:P], SQRT)

    x = T(4 + N, "x")
    nc.sync.dma_start(out=x[:], in_=xd.ap())
    f = x[:, 0:1]
    sqf = x[:, 1:2]
    ln = x[:, 2:3]
    d1 = x[:, 3:4]
    tg = x[:, 4:4 + N]

    # ---- scalar ramp (G, off the V critical chain) ------------------------
    dd = T(1, "dd")
    V.tensor_scalar(dd[:], d1, 5e-4, -D0 / 2000.0, MULT, ADD)
    dL2 = T(1, "dL2")
    G.tensor_scalar(dL2[:], ln, 2.0 * math.pi / C_SOUND / 12800.0, None, MULT)
    dL3 = T(1, "dL3")
    G.tensor_scalar(dL3[:], ln, 3e-5 / 12800.0, None, MULT)
    y = T(1, "y")
    G.tensor_mul(y[:], f, dL2[:])
    s_ = T(1, "s_")
    G.tensor_mul(s_[:], sqf, dL3[:])
    s1 = T(1, "s1")
    G.tensor_scalar(s1[:], s_[:], K1, None, MULT)
    y1 = T(1, "y1")
    G.tensor_scalar(y1[:], y[:], K1, None, MULT)
    s2 = T(1, "s2")
    G.tensor_scalar(s2[:], s_[:], K2, None, MULT)
    y2 = T(1, "y2")
    G.tensor_scalar(y2[:], y[:], K2, None, MULT)
    y_sq = T(1, "y_sq")
    G.tensor_mul(y_sq[:], y[:], y[:])
    sy = T(1, "sy")
    G.tensor_mul(sy[:], s_[:], y[:])
    ydd = T(1, "ydd")
    G.tensor_mul(ydd[:], y_sq[:], dd[:])
    c_dre = T(1, "c_dre")
    G.tensor_scalar(c_dre[:], ydd[:], -21.0 / 128.0, None, MULT)
    sdd = T(1, "sdd")
    G.tensor_mul(sdd[:], sy[:], dd[:])
    c_dim = T(1, "c_dim")
    G.tensor_scalar(c_dim[:], sdd[:], 5.25 / 128.0, None, MULT)
    # radiation load ZL
    r_end = T(1, "r_end")
    G.tensor_scalar(r_end[:], d1, 5e-4, None, MULT)
    rei = T(1, "rei")
    V.reciprocal(rei[:], r_end[:])
    z0e = T(1, "z0e")
    G.tensor_mul(z0e[:], rei[:], rei[:])
    re2 = T(1, "re2")
    G.tensor_scalar(re2[:], r_end[:], 2.0 * math.pi / C_SOUND, None, MULT)
    zkr = T(1, "zkr")
    G.tensor_mul(zkr[:], f, re2[:])
    zkr2 = T(1, "zkr2")
    G.tensor_mul(zkr2[:], zkr[:], zkr[:])
    z0a = T(1, "z0a")
    G.tensor_scalar(z0a[:], z0e[:], 0.25 * K1, None, MULT)
    z0b = T(1, "z0b")
    G.tensor_scalar(z0b[:], z0e[:], 0.61 * K1, None, MULT)
    zlre = T(1, "zlre")
    S.activation(zlre[:], zkr2[:], COPY, scale=z0a[:])
    zlim = T(1, "zlim")
    S.activation(zlim[:], zkr[:], COPY, scale=z0b[:])
    zlimN = T(1, "zlimN")
    G.tensor_scalar(zlimN[:], zlim[:], -1.0, None, MULT)

    # ---- slice stage: QT = [s1*ri^3 | y1*ri^2 | s2*r | y2*r^2 | rinv] -----
    RT = T(N, "RT")
    V.tensor_scalar(RT[:], tg, dd[:], D0 / 2000.0, MULT, ADD)              # r
    QT = T(5 * N, "QT")
    V.reciprocal(QT[:, 4 * N:5 * N], RT[:])                                # rinv
    RI2 = T(N, "RI2")
    V.tensor_mul(RI2[:], QT[:, 4 * N:5 * N], QT[:, 4 * N:5 * N])
    V.tensor_scalar(QT[:, 2 * N:3 * N], RT[:], s2[:], None, MULT)          # s2*r
    V.scalar_tensor_tensor(QT[:, N:2 * N], QT[:, 4 * N:5 * N], y1[:],
                           QT[:, 4 * N:5 * N], MULT, MULT)                 # y1*ri^2
    V.scalar_tensor_tensor(QT[:, 0:N], RI2[:], s1[:],
                           QT[:, 4 * N:5 * N], MULT, MULT)                 # s1*ri^3
    R2T = T(N, "R2T")
    G.tensor_mul(R2T[:], RT[:], RT[:])
    G.tensor_scalar(QT[:, 3 * N:4 * N], R2T[:], y2[:], None, MULT)         # y2*r^2

    # ---- group sums: LT6[0:64] = [b_re|b_im|c_re|c_im] ; R5 = sum rinv ----
    LT6 = T(96, "LT6")
    V.tensor_reduce(LT6[:, 0:64], ap(QT, 0, [[8, 64], [1, 8]]), X_AX, ADD)
    R5 = T(16, "R5")
    V.tensor_reduce(R5[:], ap(QT, 4 * N, [[8, 16], [1, 8]]), X_AX, ADD)
    # d terms (G): d_re = c_dre*R5 ; d_im = c_dim*R5^2
    G.tensor_scalar(LT6[:, 64:80], R5[:], c_dre[:], None, MULT)
    R5sq = T(16, "R5sq")
    G.tensor_mul(R5sq[:], R5[:], R5[:])
    G.tensor_scalar(LT6[:, 80:96], R5sq[:], c_dim[:], None, MULT)

    # ---- w = b*c packed [w_re|w_im] ---------------------------------------
    P1 = T(32, "P1")
    V.tensor_tensor(P1[:], LT6[:, 0:32], LT6[:, 32:64], MULT)
    P2 = T(32, "P2")
    V.tensor_tensor(P2[:], LT6[:, 0:32], ap(LT6, 48, [[-16, 2], [1, 16]]), MULT)
    WP = T(32, "WP")
    V.tensor_sub(WP[:, 0:16], P1[:, 0:16], P1[:, 16:32])
    V.tensor_add(WP[:, 16:32], P2[:, 0:16], P2[:, 16:32])

    # ---- series prep (parallel octet on V/G/Act) --------------------------
    # C = u1 + w^2*Bc, S = v1 + w^2*Bs ; u1 = 1+w/2, v1 = 1+w/6,
    # Bc = 1/24 + w/720, Bs = 1/120 + w/5040
    wre = WP[:, 0:16]
    wim = WP[:, 16:32]
    SQt = T(32, "SQt")
    V.tensor_tensor(SQt[:], WP[:], WP[:], MULT)
    W2D = T(64, "W2D")                    # [w2re|w2im|w2im|w2re]
    V.scalar_tensor_tensor(ap(W2D, 16, [[16, 2], [1, 16]]),
                           ap(WP, 0, [[0, 2], [1, 16]]), 2.0,
                           ap(WP, 16, [[0, 2], [1, 16]]), MULT, MULT)
    V.tensor_tensor(ap(W2D, 0, [[48, 2], [1, 16]]),
                    ap(SQt, 0, [[0, 2], [1, 16]]),
                    ap(SQt, 16, [[0, 2], [1, 16]]), SUB)
    U12 = T(96, "U12")                    # [u1re|u1im|v1re|v1im|v1im|v1re]
    V.tensor_scalar(U12[:, 0:16], wre, 0.5, 1.0, MULT, ADD)
    S.activation(U12[:, 16:32], wim, COPY, scale=0.5)
    G.tensor_scalar(ap(U12, 32, [[48, 2], [1, 16]]),
                    ap(WP, 0, [[0, 2], [1, 16]]), 1.0 / 6.0, 1.0, MULT, ADD)
    G.tensor_scalar(ap(U12, 48, [[16, 2], [1, 16]]),
                    ap(WP, 16, [[0, 2], [1, 16]]), 1.0 / 6.0, None, MULT)
    BP = T(64, "BP")                      # [Bcre|Bcim|Bsre|Bsim]
    V.tensor_scalar(BP[:, 0:16], wre, 1.0 / 720.0, 1.0 / 24.0, MULT, ADD)
    S.activation(BP[:, 16:32], wim, COPY, scale=1.0 / 720.0)
    G.tensor_scalar(BP[:, 32:48], wre, 1.0 / 5040.0, 1.0 / 120.0, MULT, ADD)
    S.activation(BP[:, 48:64], wim, COPY, scale=1.0 / 5040.0)


    # ---- m = w^2 * B (complex, packed for C and S at once) ----------------
    T1 = T(64, "T1")
    V.tensor_tensor(T1[:], BP[:], ap(W2D, 0, [[0, 2], [1, 32]]), MULT)
    T2 = T(64, "T2")
    V.tensor_tensor(T2[:], BP[:], ap(W2D, 32, [[0, 2], [1, 32]]), MULT)
    M = T(64, "M")                        # [mCre|mCim|mSre|mSim]
    V.tensor_tensor(ap(M, 0, [[32, 2], [1, 16]]),
                    ap(T1, 0, [[32, 2], [1, 16]]),
                    ap(T1, 16, [[32, 2], [1, 16]]), SUB)
    V.tensor_tensor(ap(M, 16, [[32, 2], [1, 16]]),
                    ap(T2, 0, [[32, 2], [1, 16]]),
                    ap(T2, 16, [[32, 2], [1, 16]]), ADD)
    CTS = T(96, "CTS")                    # [Cre|Cim|Sre|Sim|Sim|Sre]
    V.tensor_tensor(CTS[:, 0:64], U12[:, 0:64], M[:], ADD)
    V.tensor_tensor(CTS[:, 64:96], U12[:, 64:96],
                    ap(M, 48, [[-16, 2], [1, 16]]), ADD)
    # ---- entries V0 = re(A,B,C,D)|im(A,B,C,D), NX0 = negated im -----------
    V0 = T(128, "V0")
    NX = T(64, "NX0")
    PP1 = T(96, "PP1")
    V.tensor_tensor(PP1[:], LT6[:], ap(CTS, 32, [[0, 3], [1, 32]]), MULT)
    PP2 = T(96, "PP2")
    V.tensor_tensor(PP2[:], LT6[:], ap(CTS, 64, [[0, 3], [1, 32]]), MULT)
    V.tensor_tensor(ap(V0, 16, [[16, 3], [1, 16]]),
                    ap(PP1, 0, [[32, 3], [1, 16]]),
                    ap(PP1, 16, [[32, 3], [1, 16]]), SUB)
    V.tensor_tensor(ap(V0, 80, [[16, 3], [1, 16]]),
                    ap(PP2, 0, [[32, 3], [1, 16]]),
                    ap(PP2, 16, [[32, 3], [1, 16]]), ADD)
    V.tensor_add(V0[:, 0:16], CTS[:, 0:16], V0[:, 48:64])       # A_re
    V.tensor_sub(V0[:, 48:64], CTS[:, 0:16], V0[:, 48:64])      # D_re
    V.tensor_add(V0[:, 64:80], CTS[:, 16:32], V0[:, 112:128])   # A_im
    # negated-im tile (all V; must read dSim before D_im's in-place write)
    V.tensor_scalar(NX[:, 16:48], V0[:, 80:112], -1.0, None, MULT)  # -B,-C im
    V.scalar_tensor_tensor(NX[:, 0:16], CTS[:, 16:32], -1.0,
                           V0[:, 112:128], MULT, SUB)           # -A_im
    V.tensor_sub(NX[:, 48:64], V0[:, 112:128], CTS[:, 16:32])   # -D_im
    V.tensor_sub(V0[:, 112:128], CTS[:, 16:32], V0[:, 112:128])  # D_im

    # ---- 3 generic tree levels (16 -> 2 matrices) -------------------------
    X = V0
    n = 16
    lvl = 0
    while n > 2:
        m = n // 2
        lvl += 1
        io = 4 * n
        U = T(32 * m, f"U{lvl}")
        Xn = T(8 * m, f"X{lvl}")

        def uo(c, k, s_i):
            return ap(U, c * 16 * m + 2 * k + s_i, [[8 * m, 2], [4 * m, 2], [4, m]])

        def li(k, imag):
            return ap(X, k * n + (io if imag else 0), [[2 * n, 2], [0, 2], [2, m]])

        def lin(k):
            return ap(NX, k * n, [[2 * n, 2], [0, 2], [2, m]])

        def ri(k, imag):
            return ap(X, 2 * k * n + 1 + (io if imag else 0), [[0, 2], [n, 2], [2, m]])

        V.tensor_tensor(uo(0, 0, 0), li(0, 0), ri(0, 0), MULT)
        V.tensor_tensor(uo(0, 1, 0), li(1, 0), ri(1, 0), MULT)
        V.tensor_tensor(uo(0, 0, 1), lin(0), ri(0, 1), MULT)
        G.tensor_tensor(uo(0, 1, 1), lin(1), ri(1, 1), MULT)
        V.tensor_tensor(uo(1, 0, 0), li(0, 0), ri(0, 1), MULT)
        G.tensor_tensor(uo(1, 1, 0), li(1, 0), ri(1, 1), MULT)
        V.tensor_tensor(uo(1, 0, 1), li(0, 1), ri(0, 0), MULT)
        G.tensor_tensor(uo(1, 1, 1), li(1, 1), ri(1, 0), MULT)
        V.tensor_reduce(Xn[:], ap(U, 0, [[4, 8 * m], [1, 4]]), X_AX, ADD)
        if m > 2:
            NXn = T(4 * m, f"NX{lvl}")
            V.tensor_reduce(NXn[:], ap(U, 16 * m, [[4, 4 * m], [1, 4]]),
                            X_AX, ADD, negate=True)
            NX = NXn
        X = Xn
        n = m

    # ---- final combine with ZL folded in ----------------------------------
    # X [P,16]: blocks of 2 (L=idx0, R=idx1): [Are|Bre|Cre|Dre|Aim|Bim|Cim|Dim]
    # R' = R.[ZL;1]: R11' = R11*ZL + R12 ; R21' = R21*ZL + R22 (stt-fused)
    rre = ap(X, 1, [[4, 2]])              # [R11re, R21re]
    rim = ap(X, 9, [[4, 2]])              # [R11im, R21im]
    ST1 = T(4, "ST1")
    V.scalar_tensor_tensor(ST1[:, 0:2], rim, zlimN[:], ap(X, 3, [[4, 2]]),
                           MULT, ADD)     # -Zi*Rim + R12/22re
    V.scalar_tensor_tensor(ST1[:, 2:4], rre, zlim[:], ap(X, 11, [[4, 2]]),
                           MULT, ADD)     # Zi*Rre + R12/22im
    RP = T(4, "RP")                       # [R11're, R21're, R11'im, R21'im]
    V.scalar_tensor_tensor(RP[:, 0:2], rre, zlre[:], ST1[:, 0:2], MULT, ADD)
    V.scalar_tensor_tensor(RP[:, 2:4], rim, zlre[:], ST1[:, 2:4], MULT, ADD)
    # products: [num|den] = [A*R11'+B*R21' | C*R11'+D*R21'] (complex)
    U5 = T(16, "U5")
    lre = ap(X, 0, [[2, 4]])              # [ALre, BLre, CLre, DLre]
    lim = ap(X, 8, [[2, 4]])
    rpre = ap(RP, 0, [[0, 2], [1, 2]])    # [R11're, R21're] x2
    rpim = ap(RP, 2, [[0, 2], [1, 2]])
    V.tensor_tensor(ap(U5, 0, [[4, 2], [1, 2]]), lre, rpre, MULT)
    V.scalar_tensor_tensor(ap(U5, 2, [[4, 2], [1, 2]]), lim, -1.0, rpim, MULT, MULT)
    G.tensor_tensor(ap(U5, 8, [[4, 2], [1, 2]]), lre, rpim, MULT)
    G.tensor_tensor(ap(U5, 10, [[4, 2], [1, 2]]), lim, rpre, MULT)
    nd = T(4, "nd")                       # [num_re|den_re|num_im|den_im]
    V.tensor_reduce(nd[:], ap(U5, 0, [[4, 4], [1, 4]]), X_AX, ADD)
    sq = T(4, "sq")
    V.tensor_mul(sq[:], nd[:], nd[:])
    ND2 = T(2, "ND2")
    V.tensor_add(ND2[:], sq[:, 0:2], sq[:, 2:4])
    rde = T(1, "rde")
    V.reciprocal(rde[:], ND2[:, 1:2])
    rat = T(1, "rat")
    V.tensor_mul(rat[:], ND2[:, 0:1], rde[:])
    res = T(1, "res")
    S.activation(res[:], rat[:], SQRT)
    nc.sync.dma_start(out=outd.ap(), in_=res[:])


def build_program(fpc, loop_iters=None, unroll=1):
    """Build the SPMD Bass program; every core runs it on its own 47 freqs.

    unroll > 1 (timing path only) emits `unroll` independent bodies per
    For_i iteration, each with its own tile set, so successive bodies
    software-pipeline across engines and the per-iteration all-engine
    barrier amortizes."""
    nc = bacc.Bacc("TRN2", target_bir_lowering=False, debug=False)
    P = fpc
    N = N_SUB

    xd = nc.dram_tensor("x", [P, 4 + N], F32, kind="ExternalInput")
    outd = nc.dram_tensor("out", [P, 1], F32, kind="ExternalOutput")

    with tile.TileContext(nc) as tc, ExitStack() as ctx:
        pool = ctx.enter_context(tc.tile_pool(name="p", bufs=1))
        if loop_iters is None:
            _emit_body(nc, tc, pool, P, xd, outd)
        else:
            with tc.For_i(0, loop_iters, 1):
                for u in range(unroll):
                    _emit_body(nc, tc, pool, P, xd, outd, sfx=f"_u{u}")

    nc.compile()
    return nc


_PROGRAM_CACHE = {}


def _get_program(fpc):
    if fpc not in _PROGRAM_CACHE:
        _PROGRAM_CACHE[fpc] = build_program(fpc)
    return _PROGRAM_CACHE[fpc]


def make_inputs(length, d1, fmin, fmax, fpc):
    """Host-side shard prep: pack [f | sqrt(f) | length | d1 | t] per core."""
    F = fmax - fmin
    f_full = np.arange(fmin, fmax, dtype=np.float32)
    f_pad = np.concatenate([f_full, np.full(N_CORES * fpc - F, float(fmin), np.float32)])
    t = ((np.arange(N_SUB, dtype=np.float32) + 0.5) / N_SUB)
    in_maps = []
    for c in range(N_CORES):
        X = np.empty((fpc, 4 + N_SUB), dtype=np.float32)
        X[:, 0] = f_pad[c * fpc:(c + 1) * fpc]
        X[:, 1] = np.sqrt(f_pad[c * fpc:(c + 1) * fpc])
        X[:, 2] = np.float32(length[0])
        X[:, 3] = np.float32(d1[0])
        X[:, 4:] = t[None, :]
        in_maps.append({"x": X})
    return in_maps


def kernel(length, d1, fmin, fmax):
    length = np.asarray(length, dtype=np.float32)
    d1 = np.asarray(d1, dtype=np.float32)
    fmin = int(fmin)
    fmax = int(fmax)
    F = fmax - fmin
    fpc = (F + N_CORES - 1) // N_CORES
    nc = _get_program(fpc)
    in_maps = make_inputs(length, d1, fmin, fmax, fpc)
    res = run_bass_kernel_spmd(nc, in_maps, list(range(N_CORES)))
    outs = [res.results[c]["out"].reshape(-1) for c in range(N_CORES)]
    return np.concatenate(outs)[:F].astype(np.float32)
